# revision 3
# baseline (speedup 1.0000x reference)
"""DeepSeek sparse attention TRN2 kernel v2: 8-core query-parallel.

Hardcoded for B=1, S=768, E=512, H=8, DK=64, TOPK=384, 8 cores.
Core c owns queries [96c, 96c+96). Output = host concat of per-core rows.

v2 vs baseline:
  - k_idx t-major (out [128t,64]; fp32 cost = N*4cyc so M=128 halves it),
    then PE-transpose to [64,768]; ikb bias folded into the transpose drain.
  - scores packed (4h x 32q) fp32; relu*w on DVE; head-half pre-adds on
    Pool; combine = 3 fp32 bd01 passes + 1 fp32 ramp pass per t-half.
  - topk: lo-only bisection (compile-time halving widths), DVE-only counts,
    exact top-16 fixup (reproduces lax.top_k boundary handling).
  - attention: K/Q head-paired (M=128), V t-major, QK per head [128t,96],
    exp on ACT (emitted before topk in ACT program order so it overlaps the
    bisection), multiplicative 0/1 mask, denominators via N=1 matmuls into
    pden [96,8], attn accumulated t-major into [96,512], single reciprocal
    + single normalize, PE transposes (no DMA round trips), paired
    out-projection; bk dropped, bv folded into bo2 on host.
"""
import numpy as np
import ml_dtypes

S, E, H, DK = 768, 512, 8, 64
NQ = 96
NC = 8
KCH = 4            # 512/128
TCH = 6            # 768/128
TH = 384
SCALING = 1.0 / np.sqrt(DK)
RAMP_EPS = float(2.0 ** -40)
R_ITERS = 8
BRK = 1.2
NEG = -1e30

# smalls column layout (f32 [128, 336])
SM_IQB = 0      # 0:4   iqb per m-chunk
SM_BQP = 4      # 4:8   bq per head-pair chunk
SM_IKB = 8      # 8     ikb (partitions 0:64)
SM_WPB = 9      # 9     wpb (partitions 0:8)
SM_ID = 16      # 16:144 identity 128x128 (f32)
SM_BD = 144     # 144:304 bd01 sliding selection matrix

# bcast row layout (f32 [1, 528] -> [96, 528])
BC_COL16 = 0    # 0:16
BC_BO2 = 16     # 16:528


def build_nc(stage=99):
    import concourse.bass as bass
    import concourse.bacc as bacc
    from concourse import mybir
    from concourse.tile import TileContext

    f32 = mybir.dt.float32
    bf16 = mybir.dt.bfloat16
    AF = mybir.ActivationFunctionType
    OP = mybir.AluOpType

    nc = bacc.Bacc("TRN2", target_bir_lowering=False, debug=False)

    def din(name, shape, dt):
        return nc.dram_tensor(name, shape, dt, kind="ExternalInput")

    xTq4 = din("xTq4", [128, KCH, NQ], f32)
    xTq16 = din("xTq16", [128, KCH, NQ], bf16)
    iqW4 = din("iqW4", [128, KCH, 4, 128], f32)
    ikW4 = din("ikW4", [128, KCH, DK], f32)
    wpW4 = din("wpW4", [128, KCH, H], f32)
    xT6 = din("xT6", [128, KCH, TCH, 128], f32)
    smalls = din("smalls", [128, 336], f32)
    rowF = din("rowF", [1, 96 + S], f32)      # ones96, nramp (f32)
    bcrow = din("bcrow", [1, 528], f32)       # col16, bo2
    x16T = din("x16T", [128, KCH, S], bf16)
    wkP = din("wkP", [128, KCH, 4, 128], bf16)
    wqP = din("wqP", [128, KCH, 4, 128], bf16)
    wvP = din("wvP", [128, KCH, E], bf16)
    woP = din("woP", [128, 4, E], bf16)
    out = nc.dram_tensor("out", [NQ, E], f32, kind="ExternalOutput")
    dbg = nc.dram_tensor("dbg", [NQ, S], f32, kind="ExternalOutput")
    wT_dram = nc.dram_tensor("wT_dram", [H, NQ], f32)

    def bcastP(ap, p):
        return bass.AP(tensor=ap.tensor, offset=ap.offset,
                       ap=[[0, p]] + ap.ap[1:])

    import contextlib
    with TileContext(nc) as tc:
      with contextlib.suppress(StopIteration):
        with tc.tile_pool(name="w1", bufs=1) as w1, \
             tc.tile_pool(name="big", bufs=1) as big, \
             tc.tile_pool(name="scp", bufs=2) as scp, \
             tc.tile_pool(name="krw", bufs=2) as krw, \
             tc.tile_pool(name="tiny", bufs=1) as tiny, \
             tc.tile_pool(name="psA", bufs=2, space="PSUM") as psA, \
             tc.tile_pool(name="psT", bufs=2, space="PSUM") as psT, \
             tc.tile_pool(name="psI", bufs=2, space="PSUM") as psI, \
             tc.tile_pool(name="psO", bufs=2, space="PSUM") as psO:

            # ---------------- SBUF weight tiles ----------------
            s_xTq = w1.tile([128, KCH, NQ], f32)
            s_xTq16 = w1.tile([128, KCH, NQ], bf16)
            s_iqW = w1.tile([128, KCH, 4, 128], f32)
            s_ikW = w1.tile([128, KCH, DK], f32)
            s_wpW = w1.tile([128, KCH, H], f32)
            s_xT = w1.tile([128, KCH, TCH, 128], f32)
            s_sm = w1.tile([128, 336], f32)
            s_rowF = w1.tile([1, 96 + S], f32)
            s_bc = w1.tile([NQ, 528], f32)
            s_x16 = w1.tile([128, KCH, S], bf16)
            s_wk = w1.tile([128, KCH, 4, 128], bf16)
            s_wq = w1.tile([128, KCH, 4, 128], bf16)
            s_wv = w1.tile([128, KCH, E], bf16)
            s_wo = w1.tile([128, 4, E], bf16)

            # loads in need-order on SP queue
            nc.sync.dma_start(out=s_xTq, in_=xTq4[:, :, :])
            nc.sync.dma_start(out=s_sm, in_=smalls[:, :])
            for m in range(4):
                nc.sync.dma_start(out=s_iqW[:, :, m, :],
                                  in_=iqW4[:, :, m, :])
            nc.sync.dma_start(out=s_ikW, in_=ikW4[:, :, :])
            nc.sync.dma_start(out=s_wpW, in_=wpW4[:, :, :])
            for t in range(TCH):
                nc.sync.dma_start(out=s_xT[:, :, t, :], in_=xT6[:, :, t, :])
            nc.sync.dma_start(out=s_rowF, in_=rowF[:, :])
            nc.sync.dma_start(out=s_bc, in_=bcastP(bcrow[:, :], NQ))
            nc.sync.dma_start(out=s_xTq16, in_=xTq16[:, :, :])
            nc.sync.dma_start(out=s_x16, in_=x16T[:, :, :])
            nc.sync.dma_start(out=s_wk, in_=wkP[:, :, :, :])
            nc.sync.dma_start(out=s_wq, in_=wqP[:, :, :, :])
            nc.sync.dma_start(out=s_wv, in_=wvP[:, :, :])
            nc.sync.dma_start(out=s_wo, in_=woP[:, :, :])

            # =========== INDEXER ===========
            # w-proj (M=8 orientation; wpb bias per-partition on drain)
            s_wT = tiny.tile([H, NQ], f32)
            pwT = psA.tile([H, NQ], f32, tag="ps")
            for k in range(KCH):
                nc.tensor.matmul(pwT, s_wpW[:, k, :], s_xTq[:, k, :],
                                 start=(k == 0), stop=(k == KCH - 1))
            nc.scalar.activation(out=s_wT, in_=pwT, func=AF.Identity,
                                 bias=s_sm[0:H, SM_WPB:SM_WPB + 1])

            # q-proj: s_qid [128(E' in chunk), m, 96]
            s_qid = big.tile([128, KCH, NQ], f32)
            for m in range(KCH):
                pq = psA.tile([128, NQ], f32, tag="ps")
                for k in range(KCH):
                    nc.tensor.matmul(pq, s_iqW[:, k, m, :], s_xTq[:, k, :],
                                     start=(k == 0), stop=(k == KCH - 1))
                nc.scalar.activation(out=s_qid[:, m, :], in_=pq,
                                     func=AF.Identity,
                                     bias=s_sm[:, SM_IQB + m:SM_IQB + m + 1])

            # score lhsT tiles: sc_all [64, 768], tile (g,hf) = cols
            # 128*(3hf+g) + 32*hl + s, hl = 2*mlo + rbit (head 4hf+hl).
            # 8 DMAs (ri,hf,mlo), each 3-dim: dst [part,[128,3],[1,32]].
            sc_all = big.tile([DK, 768], f32, name="sc_all")
            for ri, r in enumerate((0, DK)):
                for hf in range(2):
                    for mlo in range(2):
                        src_sl = s_qid[r:r + DK, 2 * hf + mlo, :]
                        src = bass.AP(tensor=src_sl.tensor,
                                      offset=src_sl.offset,
                                      ap=[src_sl.ap[0], [32, 3], [1, 32]])
                        dst = bass.AP(
                            tensor=sc_all.tensor,
                            offset=(sc_all.offset + 384 * hf
                                    + 32 * (2 * mlo + ri)),
                            ap=[sc_all.ap[0], [128, 3], [1, 32]])
                        nc.scalar.dma_start(out=dst, in_=src)
            if stage == 115:
                raise StopIteration

            def sc_lhs(g, hf):
                j = 3 * hf + g
                return sc_all[:, 128 * j:128 * (j + 1)]

            # wcolALL [128, 6] col 3*hf+g: partition 32*hl+s <- wT[4hf+hl,
            # 32g+s]. SBUF->SBUF can't reshape partitions, so round-trip
            # the tiny [8,96] wT through DRAM (write + 2 pattern reads).
            wcolALL = tiny.tile([128, 6], f32, name="wcolALL")
            nc.scalar.dma_start(out=wT_dram[:, :], in_=s_wT)
            wT_base = wT_dram[:, :]
            for hf in range(2):
                for hl in range(4):
                    src = bass.AP(tensor=wT_base.tensor,
                                  offset=wT_base.offset + (4 * hf + hl) * NQ,
                                  ap=[[1, 32], [32, 3]])
                    dst_sl = wcolALL[32 * hl:32 * (hl + 1),
                                     3 * hf:3 * hf + 3]
                    dst = bass.AP(tensor=dst_sl.tensor, offset=dst_sl.offset,
                                  ap=[dst_sl.ap[0], [1, 3]])
                    nc.scalar.dma_start(out=dst, in_=src)
            if stage == 116:
                raise StopIteration

            # k-proj t-major + PE transpose -> s_kT [64, 6, 128] (+ikb bias)
            s_kT = big.tile([DK, TCH, 128], f32)
            for t in range(TCH):
                pk = psA.tile([128, DK], f32, tag="ps")
                for k in range(KCH):
                    nc.tensor.matmul(pk, s_xT[:, k, t, :], s_ikW[:, k, :],
                                     start=(k == 0), stop=(k == KCH - 1))
                s_kraw = krw.tile([128, DK], f32, tag="kraw")
                nc.vector.tensor_copy(s_kraw, pk)
                ptk = psA.tile([DK, 128], f32, tag="ps")
                nc.tensor.transpose(ptk, s_kraw,
                                    s_sm[:, SM_ID:SM_ID + 128])
                nc.vector.tensor_scalar_add(
                    s_kT[:, t, :], ptk, s_sm[0:DK, SM_IKB:SM_IKB + 1])

            if stage == 11:
                s_oA = big.tile([NQ, S], f32, name="s_oA")
                nc.vector.memset(s_oA, 0.0)
                nc.vector.tensor_copy(s_oA[0:DK, 0:128], s_kT[:, 0, :])
                nc.vector.tensor_copy(s_oA[:, 128:224], s_qid[0:NQ, 0, :])
                nc.vector.tensor_copy(s_oA[0:H, 224:320], s_wT)
                nc.sync.dma_start(out=dbg[:, :], in_=s_oA)
                raise StopIteration

            # scores (fp32 packed) + relu*w (DVE) + hf-preadd (Pool)
            wsum = [[scp.tile([128, TH], f32, tag=f"wsum_{g}_{th}",
                              name=f"wsum_{g}_{th}")
                     for th in range(2)] for g in range(3)]
            for th in range(2):
                for g in range(3):
                    ws01 = []
                    for hf in range(2):
                        psc = psA.tile([128, TH], f32, tag="ps")
                        nc.tensor.matmul(psc, sc_lhs(g, hf),
                                         s_kT[:, 3 * th:3 * th + 3, :],
                                         start=True, stop=True)
                        wst = scp.tile([128, TH], f32, tag=f"ws_{hf}",
                                       name=f"ws_{g}_{hf}_{th}")
                        nc.vector.scalar_tensor_tensor(
                            out=wst, in0=psc, scalar=0.0,
                            in1=wcolALL[:, 3 * hf + g:3 * hf + g + 1
                                        ].to_broadcast([128, TH]),
                            op0=OP.max, op1=OP.mult)
                        ws01.append(wst)
                    nc.gpsimd.tensor_add(wsum[g][th], ws01[0], ws01[1])

            if stage in (117, 118):
                raise StopIteration

            # combine: pind[96,384] = sum_g bd01_g^T wsum_g + f32 ramp pass
            s_ind = big.tile([NQ, S], f32)
            for th in range(2):
                pind = psI.tile([NQ, TH], f32, tag="pind")
                for g in range(3):
                    nc.tensor.matmul(
                        pind,
                        s_sm[:, SM_BD + 64 - 32 * g:SM_BD + 160 - 32 * g],
                        wsum[g][th], start=(g == 0), stop=False)
                nc.tensor.matmul(pind, s_rowF[:, 0:NQ],
                                 s_rowF[:, 96 + TH * th:96 + TH * (th + 1)],
                                 start=False, stop=True)
                nc.scalar.copy(s_ind[:, TH * th:TH * (th + 1)], pind)

            if stage == 12:
                nc.sync.dma_start(out=dbg[:, :], in_=s_ind)
                s_o0 = big.tile([NQ, E], f32, name="s_o0")
                nc.vector.memset(s_o0, 0.0)
                nc.sync.dma_start(out=out[:, :], in_=s_o0)
                raise StopIteration

            # ====== ATTENTION PROJECTIONS + QK + EXP (overlap topk) ======
            s_KT = big.tile([128, 4, S], bf16)
            s_QT = big.tile([128, 4, NQ], bf16)
            s_V = big.tile([128, TCH, E], bf16)
            for hp in range(4):
                for th in range(2):
                    pK = psA.tile([128, TH], f32, tag="ps")
                    for k in range(KCH):
                        nc.tensor.matmul(pK, s_wk[:, k, hp, :],
                                         s_x16[:, k, TH * th:TH * (th + 1)],
                                         start=(k == 0), stop=(k == KCH - 1))
                    nc.scalar.copy(
                        s_KT[:, hp, TH * th:TH * (th + 1)], pK)
                pQ = psA.tile([128, NQ], f32, tag="ps")
                for k in range(KCH):
                    nc.tensor.matmul(pQ, s_wq[:, k, hp, :],
                                     s_xTq16[:, k, :],
                                     start=(k == 0), stop=(k == KCH - 1))
                nc.scalar.activation(
                    out=s_QT[:, hp, :], in_=pQ, func=AF.Identity,
                    bias=s_sm[:, SM_BQP + hp:SM_BQP + hp + 1])

            # QK + exp (ACT) — before V so exp tiles appear early
            w_tiles = [[scp.tile([128, 4 * NQ], bf16, tag=f"wt_{t}_{q}",
                                 name=f"wt_{t}_{q}") for q in range(2)]
                       for t in range(TCH)]
            for t in range(TCH):
                for q in range(2):
                    pqk = psA.tile([128, 4 * NQ], f32, tag="ps")
                    for hl in range(4):
                        h = 4 * q + hl
                        j, hp = h % 2, h // 2
                        nc.tensor.matmul(
                            pqk[:, NQ * hl:NQ * (hl + 1)],
                            s_KT[DK * j:DK * (j + 1), hp,
                                 128 * t:128 * (t + 1)],
                            s_QT[DK * j:DK * (j + 1), hp, :],
                            start=True, stop=True)
                    nc.scalar.activation(out=w_tiles[t][q], in_=pqk,
                                         func=AF.Exp, scale=SCALING)

            # V (t-major)
            for t in range(TCH):
                pV = psA.tile([128, E], f32, tag="ps")
                for k in range(KCH):
                    nc.tensor.matmul(pV, s_x16[:, k, 128 * t:128 * (t + 1)],
                                     s_wv[:, k, :],
                                     start=(k == 0), stop=(k == KCH - 1))
                nc.scalar.copy(s_V[:, t, :], pV)

            if stage == 118:
                raise StopIteration

            # =========== TOPK (lo-only bisection, DVE-only counts) ========
            u8 = mybir.dt.uint8
            lo = tiny.tile([NQ, 1], f32)
            mid = tiny.tile([NQ, 1], f32)
            cnt = tiny.tile([NQ, 1], f32)
            sel = tiny.tile([NQ, 1], u8)
            scrap = big.tile([NQ, S], bf16, tag="scrap")

            rsum = tiny.tile([NQ, 1], f32, name="rsum")
            hicut = big.tile([NQ, S], f32, tag="hicut")
            nc.scalar.activation(out=hicut, in_=s_ind, func=AF.Identity,
                                 bias=0.0, accum_out=rsum)
            if stage == 1308:
                raise StopIteration
            nc.vector.tensor_scalar(lo, rsum, 1.0 / S, -BRK, op0=OP.mult,
                                    op1=OP.add)
            if stage == 1310:
                raise StopIteration

            for r in range(R_ITERS):
                whalf = BRK * (2.0 ** -r)   # w_r/2, w_0 = 2*BRK
                nc.vector.tensor_scalar_add(mid, lo, whalf)
                if stage == 1311 and r == 0:
                    raise StopIteration
                nc.vector.tensor_scalar(scrap, s_ind, mid, None,
                                        op0=OP.is_ge, op1=OP.add,
                                        accum_out=cnt)
                if stage == 1312 and r == 0:
                    raise StopIteration
                nc.vector.tensor_scalar(sel, cnt, float(TH), None,
                                        op0=OP.is_ge)
                nc.vector.copy_predicated(lo, sel, mid)
                if stage == 1313 and r == 0:
                    raise StopIteration

            if stage == 131:
                raise StopIteration
            hi = tiny.tile([NQ, 1], f32)
            nc.vector.tensor_scalar_add(hi, lo,
                                        BRK * (2.0 ** -(R_ITERS - 1)))

            # exact count at hi + top-16 fixup
            c_hi = tiny.tile([NQ, 1], f32)
            nc.vector.tensor_scalar(scrap, s_ind, hi, None, op0=OP.is_ge,
                                    op1=OP.add, accum_out=c_hi)
            negbig = tiny.tile([NQ, 1], f32, name="negbig")
            nc.vector.memset(negbig, NEG)
            nc.vector.scalar_tensor_tensor(
                out=hicut, in0=s_ind, scalar=hi,
                in1=negbig.to_broadcast([NQ, S]), op0=OP.is_ge, op1=OP.mult)
            mlo = big.tile([NQ, S], f32, tag="mlo")
            nc.vector.tensor_add(mlo, hicut, s_ind)
            m16 = tiny.tile([NQ, 16], f32)
            mlo2 = big.tile([NQ, S], f32, tag="mlo2")
            nc.vector.max(out=m16[:, 0:8], in_=mlo)
            nc.vector.match_replace(out=mlo2, in_to_replace=m16[:, 0:8],
                                    in_values=mlo, imm_value=NEG)
            nc.vector.max(out=m16[:, 8:16], in_=mlo2)
            need_m1 = tiny.tile([NQ, 1], f32)
            nc.vector.tensor_scalar(need_m1, c_hi, -1.0, 383.0, op0=OP.mult,
                                    op1=OP.add)
            oh = tiny.tile([NQ, 16], f32)
            oh2 = tiny.tile([NQ, 16], f32)
            tstar = tiny.tile([NQ, 1], f32)
            nc.vector.tensor_scalar(oh, s_bc[:, BC_COL16:BC_COL16 + 16],
                                    need_m1, None, op0=OP.is_equal)
            nc.vector.scalar_tensor_tensor(out=oh2, in0=m16, scalar=1.0,
                                           in1=oh, op0=OP.mult, op1=OP.mult,
                                           accum_out=tstar)
            mask01 = big.tile([NQ, S], f32, tag="mask01")
            nc.vector.tensor_scalar(mask01, s_ind, tstar, None, op0=OP.is_ge)

            if stage == 132:
                raise StopIteration
            # maskT via PE transpose (bf16) + ACT drains
            s_maskT = big.tile([128, TCH, NQ], bf16)
            for t in range(TCH):
                ptm = psT.tile([128, NQ], f32, tag="pst")
                nc.tensor.transpose(ptm, mask01[:, 128 * t:128 * (t + 1)],
                                    s_sm[0:NQ, SM_ID:SM_ID + NQ])
                nc.scalar.copy(s_maskT[:, t, :], ptm)

            if stage == 13:
                nc.sync.dma_start(out=dbg[:, :], in_=mask01)
                s_o1 = big.tile([NQ, E], f32, name="s_o1")
                nc.vector.memset(s_o1, 0.0)
                nc.sync.dma_start(out=out[:, :], in_=s_o1)
                raise StopIteration

            # mask multiply (DVE, bf16)
            for t in range(TCH):
                msl = s_maskT[:, t, :]
                mrep = bass.AP(tensor=msl.tensor, offset=msl.offset,
                               ap=[msl.ap[0], [0, 4]] + msl.ap[1:])
                for q in range(2):
                    wt = w_tiles[t][q]
                    nc.vector.tensor_mul(wt, wt, mrep)

            # den + attn (t-major accumulation)
            s_ones128 = tiny.tile([128, 1], bf16)
            nc.vector.memset(s_ones128, 1.0)
            pden_t = psI.tile([NQ, TH], f32, tag="pind")
            pden = pden_t[:, 0:H]
            pattn = psO.tile([NQ, E], f32, tag="tail")
            for h in range(H):
                q, hl = h // 4, h % 4
                for t in range(TCH):
                    nc.tensor.matmul(pden[:, h:h + 1],
                                     w_tiles[t][q][:, NQ * hl:NQ * (hl + 1)],
                                     s_ones128,
                                     start=(t == 0), stop=(t == TCH - 1))
                for t in range(TCH):
                    nc.tensor.matmul(pattn[:, DK * h:DK * (h + 1)],
                                     w_tiles[t][q][:, NQ * hl:NQ * (hl + 1)],
                                     s_V[:, t, DK * h:DK * (h + 1)],
                                     start=(t == 0), stop=(t == TCH - 1))

            s_rcp = tiny.tile([NQ, H], f32)
            nc.vector.reciprocal(s_rcp, pden)
            s_attnT = big.tile([NQ, E], f32)
            rrep = bass.AP(tensor=s_rcp.tensor, offset=s_rcp.offset,
                           ap=[s_rcp.ap[0], [1, H], [0, DK]])
            nc.vector.tensor_mul(s_attnT, pattn, rrep)

            # attnT -> [128, 4, 96] via PE transpose + ACT drains
            s_attn = big.tile([128, 4, NQ], bf16)
            for p in range(4):
                pta = psT.tile([128, NQ], f32, tag="pst")
                nc.tensor.transpose(pta, s_attnT[:, 128 * p:128 * (p + 1)],
                                    s_sm[0:NQ, SM_ID:SM_ID + NQ])
                nc.scalar.copy(s_attn[:, p, :], pta)

            # out-projection (head-pair chunks) + bias
            po = psO.tile([NQ, E], f32, tag="tail")
            for p in range(4):
                nc.tensor.matmul(po, s_attn[:, p, :], s_wo[:, p, :],
                                 start=(p == 0), stop=(p == 3))
            s_out = big.tile([NQ, E], f32)
            nc.vector.tensor_add(s_out, po, s_bc[:, BC_BO2:BC_BO2 + E])
            nc.sync.dma_start(out=out[:, :], in_=s_out)

    nc.finalize()
    return nc


_NC_CACHE = {}


def _get_nc():
    if "nc" not in _NC_CACHE:
        _NC_CACHE["nc"] = build_nc()
    return _NC_CACHE["nc"]


def prep_inputs(x, Wq, bq_, Wk, bk_, Wv, bv_, Wo, bo_, iq_W, iq_b, ik_W, ik_b,
                wp_W, wp_b):
    bf = ml_dtypes.bfloat16
    f32 = np.float32
    xf = np.ascontiguousarray(np.asarray(x).reshape(S, E).astype(f32))
    xT = np.ascontiguousarray(xf.T)          # [E, S]

    def pack4(w, ncols):  # [E, ncols] -> [128, KCH, ncols]
        return np.ascontiguousarray(
            np.asarray(w, f32).reshape(KCH, 128, ncols).transpose(1, 0, 2))

    sm = np.zeros((128, 336), f32)
    sm[:, SM_IQB:SM_IQB + 4] = np.asarray(iq_b, f32).reshape(KCH, 128).T
    sm[:, SM_BQP:SM_BQP + 4] = np.asarray(bq_, f32).reshape(4, 128).T
    sm[0:DK, SM_IKB] = np.asarray(ik_b, f32)
    sm[0:H, SM_WPB] = np.asarray(wp_b, f32)
    sm[:, SM_ID:SM_ID + 128] = np.eye(128, dtype=f32)
    bd = np.zeros((128, 160), f32)
    for hl in range(4):
        for s_ in range(32):
            bd[32 * hl + s_, 64 + s_] = 1.0
    sm[:, SM_BD:SM_BD + 160] = bd

    rowf = np.zeros((1, 96 + S), f32)
    rowf[0, :96] = 1.0
    rowf[0, 96:] = (-np.arange(S, dtype=np.float64) * RAMP_EPS).astype(f32)

    bc = np.zeros((1, 528), f32)
    bc[0, BC_COL16:BC_COL16 + 16] = np.arange(16, dtype=f32)
    bc[0, BC_BO2:BC_BO2 + E] = (
        np.asarray(bv_, np.float64) @ np.asarray(Wo, np.float64)
        + np.asarray(bo_, np.float64)).astype(f32)

    shared = {
        "iqW4": np.ascontiguousarray(
            np.asarray(iq_W, f32).reshape(KCH, 128, KCH, 128
                                          ).transpose(1, 0, 2, 3)),
        "ikW4": pack4(ik_W, DK),
        "wpW4": pack4(wp_W, H),
        "xT6": np.ascontiguousarray(
            xT.reshape(KCH, 128, TCH, 128).transpose(1, 0, 2, 3)),
        "smalls": sm,
        "rowF": rowf,
        "bcrow": bc,
        "x16T": pack4(xT, S).astype(bf),
        "wkP": np.ascontiguousarray(
            np.asarray(Wk, f32).reshape(KCH, 128, 4, 128
                                        ).transpose(1, 0, 2, 3)).astype(bf),
        "wqP": np.ascontiguousarray(
            np.asarray(Wq, f32).reshape(KCH, 128, 4, 128
                                        ).transpose(1, 0, 2, 3)).astype(bf),
        "wvP": pack4(Wv, E).astype(bf),
        "woP": np.ascontiguousarray(
            np.asarray(Wo, f32).reshape(4, 128, E).transpose(1, 0, 2)
        ).astype(bf),
    }
    in_maps = []
    for c in range(NC):
        m = dict(shared)
        xq = np.ascontiguousarray(xT[:, NQ * c:NQ * (c + 1)])
        m["xTq4"] = np.ascontiguousarray(
            xq.reshape(KCH, 128, NQ).transpose(1, 0, 2))
        m["xTq16"] = m["xTq4"].astype(bf)
        in_maps.append(m)
    return in_maps


def kernel(**inputs):
    from concourse.bass_utils import run_bass_kernel_spmd
    nc = _get_nc()
    in_maps = prep_inputs(
        inputs["x"], inputs["Wq"], inputs["bq"], inputs["Wk"], inputs["bk"],
        inputs["Wv"], inputs["bv"], inputs["Wo"], inputs["bo"],
        inputs["iq_W"], inputs["iq_b"], inputs["ik_W"], inputs["ik_b"],
        inputs["wp_W"], inputs["wp_b"])
    res = run_bass_kernel_spmd(nc, in_maps, core_ids=list(range(NC)))
    outs = [res.results[c]["out"] for c in range(NC)]
    return np.concatenate(outs, axis=0)[None].astype(np.float32)
